# revision 1
# baseline (speedup 1.0000x reference)
"""GAT (3-layer GATConv + BatchNorm + ELU) on Trainium2, 8 NeuronCores.

Sharding: destination-node range partitioning. Core k owns dst nodes
[k*NPC, (k+1)*NPC). Dense phases (h = x @ W) are replicated on every core;
edge aggregation is sharded by dst. Layer outputs are exchanged with an
on-device AllGather (transposed, bf16); BatchNorm statistics with an
AllReduce.

Edge phase: edges are sorted by dst and processed in chunks of 128 within
blocks of 128 destination nodes. For each chunk: indirect-DMA gather of
source-node rows, exp(leaky_relu(a_s[src]+a_d[dst])) edge weights, a one-hot
(edge -> local dst) matrix built on-chip with an is_equal compare against an
iota, and a TensorE matmul onehot^T @ (ex * h_src) accumulating into PSUM.
The softmax normalization (and the /H head mean) happens after aggregation:
out[d] = (sum_e ex_e h_src) / (sum_e ex_e).
"""

import sys

sys.path.insert(0, "/opt/trn_rl_repo")

import numpy as np
from contextlib import ExitStack

import concourse.bass as bass
import concourse.mybir as mybir
import concourse.tile as tile
from concourse.bass_utils import run_bass_kernel_spmd

AF = mybir.ActivationFunctionType
ALU = mybir.AluOpType
DT = mybir.dt

# ---------------------------------------------------------------- config

NEG_SLOPE = 0.2
BN_EPS = 1e-5


class Cfg:
    def __init__(self, N=50000, E=400000, F_IN=64, HID=128, OUT=64, HEADS=4, P=8):
        assert N % P == 0
        self.N, self.E, self.F_IN, self.HID, self.OUT, self.HEADS, self.P = (
            N, E, F_IN, HID, OUT, HEADS, P,
        )
        self.NPC = N // P                      # nodes per core
        self.NBLK = (self.NPC + 127) // 128    # dst blocks per core
        self.dt_h = DT.bfloat16                # h-row / matmul dtype
        self.np_h = np.dtype(DT.np(self.dt_h))


# ---------------------------------------------------------------- host prep


def _edge_schedule(src, dst, cfg):
    """Sort edges by dst, split per core / per 128-dst block, pad to a
    uniform number of 128-edge chunks per block index across cores.

    Returns (meta_src, meta_dst, meta_dl) each [P][NCH, 128] int32 and the
    per-block chunk counts Mb [NBLK]."""
    P, NPC, NBLK = cfg.P, cfg.NPC, cfg.NBLK
    order = np.argsort(dst, kind="stable")
    s_s = src[order].astype(np.int64)
    d_s = dst[order].astype(np.int64)

    # global block start node ids: core k, block b starts at k*NPC + b*128
    blk_starts = (np.arange(P)[:, None] * NPC + np.arange(NBLK)[None, :] * 128)
    bounds = np.searchsorted(d_s, blk_starts.reshape(-1))
    bounds = np.append(bounds, len(d_s))
    cnt = np.diff(bounds).reshape(P, NBLK)
    # block NBLK-1 spills past NPC? no: searchsorted boundaries handle ragged
    Mb = np.maximum(1, -(-cnt // 128)).max(axis=0)  # chunks per block index
    NCH = int(Mb.sum())

    meta_src = np.zeros((P, NCH, 128), np.int32)
    meta_dst = np.zeros((P, NCH, 128), np.int32)
    meta_dl = np.full((P, NCH, 128), -1, np.int32)  # cast to f32 at pack time
    c0 = np.concatenate([[0], np.cumsum(Mb)])
    for k in range(P):
        for b in range(NBLK):
            i0 = bounds[k * NBLK + b]
            i1 = bounds[k * NBLK + b + 1] if (k * NBLK + b + 1) < P * NBLK else bounds[-1]
            n = i1 - i0
            if n == 0:
                continue
            base = k * NPC + b * 128
            ch = int(c0[b])
            sl_s = s_s[i0:i1]
            sl_d = d_s[i0:i1]
            full = meta_src[k, ch:ch + Mb[b]].reshape(-1)
            full[:n] = sl_s
            full = meta_dst[k, ch:ch + Mb[b]].reshape(-1)
            full[:n] = sl_d
            full = meta_dl[k, ch:ch + Mb[b]].reshape(-1)
            full[:n] = sl_d - base
    return meta_src, meta_dst, meta_dl, Mb


def _fold_alpha(W, a_s, a_d, heads, ch):
    """Fold per-head attention vectors into extra output columns of W.

    Returns W_ext [K, heads*ch + 2*heads] = [W | W@As | W@Ad] where
    (x @ W_ext)[:, HC + h] = alpha_src[:, h], [:, HC+H + h] = alpha_dst."""
    K = W.shape[0]
    HC = heads * ch
    Was = np.zeros((K, heads), np.float64)
    Wad = np.zeros((K, heads), np.float64)
    Wr = W.reshape(K, heads, ch).astype(np.float64)
    for h in range(heads):
        Was[:, h] = Wr[:, h, :] @ a_s[h].astype(np.float64)
        Wad[:, h] = Wr[:, h, :] @ a_d[h].astype(np.float64)
    return np.concatenate(
        [W.astype(np.float64), Was, Wad], axis=1
    ).astype(np.float32)


# ---------------------------------------------------------------- device program


def _fixup_dma_waits(nc, max_waits=1):
    """Walrus rejects DMA instructions with more than `max_waits` semaphore
    waits. Move the excess onto a NoOp on the issuing engine immediately
    before the DMA: the sequencer blocks on those sems, then enqueues the
    DMA with only the remaining waits (strictly more conservative order)."""
    cnt = 0
    for bb in nc.main_func.blocks:
        new = []
        for inst in bb.instructions:
            si = getattr(inst, "sync_info", None)
            if (
                "Branch" not in type(inst).__name__
                and si is not None
                and si.on_wait
                and len(si.on_wait) > max_waits
            ):
                extra = list(si.on_wait[:-max_waits])
                keep = list(si.on_wait[-max_waits:])
                for w in extra:
                    nop = mybir.InstNoOp(
                        name=f"I-dmaw-{cnt}",
                        sync_info=mybir.SyncInfo(on_wait=[w], on_update=[]),
                        bass_nofuse=True,
                        engine=inst.engine,
                    )
                    cnt += 1
                    new.append(nop)
                inst.sync_info = mybir.SyncInfo(
                    on_wait=keep, on_update=list(si.on_update))
            new.append(inst)
        bb.instructions[:] = new
    return cnt


def build_nc(cfg, Mb, num_cores=None, fixup=True):
    """Build the SPMD Bass program (identical instruction stream per core)."""
    P, N, NPC, NBLK = cfg.P, cfg.N, cfg.NPC, cfg.NBLK
    HEADS, HID, OUT, F_IN = cfg.HEADS, cfg.HID, cfg.OUT, cfg.F_IN
    HC = HEADS * HID
    dt_h = cfg.dt_h
    NCH = int(Mb.sum())
    f32 = DT.float32

    nc = bass.Bass(trn_type="TRN2", num_devices=(num_cores or P))

    # ---------------- parameters
    xT = nc.declare_dram_parameter("xT", [F_IN, N], dt_h, isOutput=False)
    m_src = nc.declare_dram_parameter("m_src", [NCH, 128], DT.int32, isOutput=False)
    m_dst = nc.declare_dram_parameter("m_dst", [NCH, 128], DT.int32, isOutput=False)
    m_dl = nc.declare_dram_parameter("m_dl", [NCH, 128], f32, isOutput=False)
    W1e = nc.declare_dram_parameter("W1e", [F_IN, HC + 2 * HEADS], dt_h, isOutput=False)
    W2e = nc.declare_dram_parameter("W2e", [HID, HC + 2 * HEADS], dt_h, isOutput=False)
    W3e = nc.declare_dram_parameter("W3e", [HID, OUT + 2], dt_h, isOutput=False)
    iota_i = nc.declare_dram_parameter("iota_i", [128, 128], f32, isOutput=False)
    ident = nc.declare_dram_parameter("ident", [128, 128], f32, isOutput=False)
    ones_c = nc.declare_dram_parameter("ones_c", [128, 1], f32, isOutput=False)
    ones_r = nc.declare_dram_parameter("ones_r", [1, 128], f32, isOutput=False)
    gbe = nc.declare_dram_parameter("gbe", [128, 4], f32, isOutput=False)
    b3r = nc.declare_dram_parameter("b3r", [128, OUT], f32, isOutput=False)
    out3 = nc.declare_dram_parameter("out3", [NBLK * 128, OUT], f32, isOutput=True)

    # ---------------- internal DRAM
    h1_rows = nc.dram_tensor("h1_rows", [N, HC], dt_h)
    h2_rows = nc.dram_tensor("h2_rows", [N, HC], dt_h)
    h3_rows = nc.dram_tensor("h3_rows", [N, OUT], dt_h)
    as1 = nc.dram_tensor("as1_all", [N, HEADS], f32)
    ad1 = nc.dram_tensor("ad1_all", [N, HEADS], f32)
    as2 = nc.dram_tensor("as2_all", [N, HEADS], f32)
    ad2 = nc.dram_tensor("ad2_all", [N, HEADS], f32)
    as3 = nc.dram_tensor("as3_all", [N, 1], f32)
    ad3 = nc.dram_tensor("ad3_all", [N, 1], f32)
    x1T_loc = nc.dram_tensor("x1T_loc", [HID, NPC], dt_h)
    x2T_loc = nc.dram_tensor("x2T_loc", [HID, NPC], dt_h)
    x1T_ag = nc.dram_tensor("x1T_ag", [P, HID, NPC], dt_h, addr_space="Shared")
    x2T_ag = nc.dram_tensor("x2T_ag", [P, HID, NPC], dt_h, addr_space="Shared")
    st1_in = nc.dram_tensor("st1_in", [128, 2], f32)
    st1_out = nc.dram_tensor("st1_out", [128, 2], f32, addr_space="Shared")
    st2_in = nc.dram_tensor("st2_in", [128, 2], f32)
    st2_out = nc.dram_tensor("st2_out", [128, 2], f32, addr_space="Shared")

    groups = [list(range(P))]

    # node tile lists
    def global_tiles():
        t = []
        n0 = 0
        while n0 < N:
            m = min(128, N - n0)
            t.append((n0, m))
            n0 += m
        return t

    def seg_tiles():
        t = []
        for s in range(P):
            o = 0
            while o < NPC:
                m = min(128, NPC - o)
                t.append((s, o, m))
                o += m
        return t

    LAST_M = NPC - (NBLK - 1) * 128  # rows in last dst block

    with tile.TileContext(nc, num_cores=(num_cores or P)) as tc:
        with ExitStack() as top:
            cpool = top.enter_context(tc.tile_pool(name="consts", bufs=1))
            opool = top.enter_context(tc.tile_pool(name="oreg", bufs=1))

            iota_s = cpool.tile([128, 128], f32, tag="iota")
            nc.sync.dma_start(out=iota_s[:], in_=iota_i[:])
            ident_s = cpool.tile([128, 128], f32, tag="ident")
            nc.sync.dma_start(out=ident_s[:], in_=ident[:])
            ones_s = cpool.tile([128, 1], f32, tag="ones")
            nc.sync.dma_start(out=ones_s[:], in_=ones_c[:])
            onesr_s = cpool.tile([1, 128], f32, tag="onesr")
            nc.sync.dma_start(out=onesr_s[:], in_=ones_r[:])
            gbe_s = cpool.tile([128, 4], f32, tag="gbe")
            nc.sync.dma_start(out=gbe_s[:], in_=gbe[:])
            b3r_s = cpool.tile([128, OUT], f32, tag="b3r")
            nc.sync.dma_start(out=b3r_s[:], in_=b3r[:])
            W1e_s = cpool.tile([F_IN, HC + 2 * HEADS], dt_h, tag="w1")
            nc.sync.dma_start(out=W1e_s[:], in_=W1e[:])
            W2e_s = cpool.tile([HID, HC + 2 * HEADS], dt_h, tag="w2")
            nc.sync.dma_start(out=W2e_s[:], in_=W2e[:])
            W3e_s = cpool.tile([HID, OUT + 2], dt_h, tag="w3")
            nc.sync.dma_start(out=W3e_s[:], in_=W3e[:])

            # persistent per-layer output region [128, NBLK*128] f32
            o_reg = opool.tile([128, NBLK * 128], f32, tag="oreg")

            # ---------------- dense phase -------------------------------
            def dense(layer, We_s, KIN, NO, src_ag, h_rows, as_a, ad_a, nheads):
                """h = x @ W_ext for all N nodes (replicated).

                src_ag: None -> read xT param; else gathered [P,HID,NPC]."""
                with ExitStack() as st:
                    dp = st.enter_context(
                        tc.tile_pool(name=f"d{layer}", bufs=3))
                    pp = st.enter_context(
                        tc.tile_pool(name=f"dp{layer}", bufs=4, space="PSUM"))
                    tiles = (
                        [(None, n0, m) for (n0, m) in global_tiles()]
                        if src_ag is None else seg_tiles()
                    )
                    for i, (s, o, m) in enumerate(tiles):
                        row0 = o if s is None else s * NPC + o
                        lhsT = dp.tile([KIN, 128], dt_h, tag="lhsT")
                        if src_ag is None:
                            nc.sync.dma_start(
                                out=lhsT[:, :m], in_=xT[:, o:o + m])
                        else:
                            nc.sync.dma_start(
                                out=lhsT[:, :m], in_=src_ag[s, :, o:o + m])
                        ps1 = pp.tile([128, NO], f32, space="PSUM", tag="ps1")
                        ps2 = pp.tile([128, 2 * nheads], f32, space="PSUM",
                                      tag="ps2")
                        nc.tensor.matmul(out=ps1[:m], lhsT=lhsT[:, :m],
                                         rhs=We_s[:, :NO],
                                         start=True, stop=True)
                        nc.tensor.matmul(out=ps2[:m], lhsT=lhsT[:, :m],
                                         rhs=We_s[:, NO:NO + 2 * nheads],
                                         start=True, stop=True)
                        hs = dp.tile([128, NO], dt_h, tag="hs")
                        # alternate copy engine to balance DVE/ACT
                        if i % 3 == 2:
                            nc.scalar.copy(out=hs[:m], in_=ps1[:m])
                        else:
                            nc.vector.tensor_copy(out=hs[:m], in_=ps1[:m])
                        al = dp.tile([128, 2 * nheads], f32, tag="al")
                        nc.vector.tensor_copy(out=al[:m], in_=ps2[:m])
                        nc.sync.dma_start(out=h_rows[row0:row0 + m, :],
                                          in_=hs[:m])
                        nc.sync.dma_start(out=as_a[row0:row0 + m, :],
                                          in_=al[:m, 0:nheads])
                        nc.sync.dma_start(out=ad_a[row0:row0 + m, :],
                                          in_=al[:m, nheads:2 * nheads])

            # ---------------- edge phase --------------------------------
            def edge(layer, h_rows, as_a, ad_a, nheads, C, last_cb=None):
                """Aggregate into o_reg[:, b*128:...] (f32, un-normalized ->
                normalized per block). C = per-head channels of h rows."""
                NOC = nheads * C
                with ExitStack() as st:
                    ep = st.enter_context(tc.tile_pool(name=f"e{layer}",
                                                       bufs=2))
                    pp = st.enter_context(
                        tc.tile_pool(name=f"ep{layer}", bufs=2, space="PSUM"))
                    ch0 = 0
                    for b in range(NBLK):
                        M = int(Mb[b])
                        mts = ep.tile([128, M], DT.int32, tag="mts")
                        mtd = ep.tile([128, M], DT.int32, tag="mtd")
                        mtl = ep.tile([128, M], f32, tag="mtl")
                        nc.sync.dma_start(
                            out=mts[:], in_=m_src[ch0:ch0 + M].rearrange(
                                "m p -> p m"))
                        nc.sync.dma_start(
                            out=mtd[:], in_=m_dst[ch0:ch0 + M].rearrange(
                                "m p -> p m"))
                        nc.sync.dma_start(
                            out=mtl[:], in_=m_dl[ch0:ch0 + M].rearrange(
                                "m p -> p m"))
                        G = ep.tile([128, M * NOC], dt_h, tag="G")
                        As = ep.tile([128, M * nheads], f32, tag="As")
                        Ad = ep.tile([128, M * nheads], f32, tag="Ad")
                        for c in range(M):
                            nc.gpsimd.indirect_dma_start(
                                out=G[:, c * NOC:(c + 1) * NOC],
                                out_offset=None,
                                in_=h_rows[:, :],
                                in_offset=bass.IndirectOffsetOnAxis(
                                    ap=mts[:, c:c + 1], axis=0),
                            )
                            nc.gpsimd.indirect_dma_start(
                                out=As[:, c * nheads:(c + 1) * nheads],
                                out_offset=None,
                                in_=as_a[:, :],
                                in_offset=bass.IndirectOffsetOnAxis(
                                    ap=mts[:, c:c + 1], axis=0),
                            )
                            nc.gpsimd.indirect_dma_start(
                                out=Ad[:, c * nheads:(c + 1) * nheads],
                                out_offset=None,
                                in_=ad_a[:, :],
                                in_offset=bass.IndirectOffsetOnAxis(
                                    ap=mtd[:, c:c + 1], axis=0),
                            )
                        # ex = exp(leaky_relu(as+ad)) over the whole block
                        v = ep.tile([128, M * nheads], f32, tag="v")
                        nc.vector.tensor_add(out=v[:], in0=As[:], in1=Ad[:])
                        vs = ep.tile([128, M * nheads], f32, tag="vs")
                        nc.vector.tensor_scalar_mul(out=vs[:], in0=v[:],
                                                    scalar1=NEG_SLOPE)
                        lr = ep.tile([128, M * nheads], f32, tag="lr")
                        nc.vector.tensor_max(out=lr[:], in0=v[:], in1=vs[:])
                        exf = ep.tile([128, M * nheads], f32, tag="exf")
                        nc.scalar.activation(out=exf[:], in_=lr[:], func=AF.Exp)
                        exb = ep.tile([128, M * nheads], dt_h, tag="exb")
                        nc.vector.tensor_copy(out=exb[:], in_=exf[:])

                        oh = ep.tile([128, M * 128], dt_h, tag="oh")
                        exG = ep.tile([128, M * NOC], dt_h, tag="exG")
                        u_ps = pp.tile([128, NOC], f32, space="PSUM", tag="u")
                        d_ps = pp.tile([128, nheads], f32, space="PSUM",
                                       tag="d")
                        for c in range(M):
                            nc.vector.tensor_scalar(
                                out=oh[:, c * 128:(c + 1) * 128],
                                in0=iota_s[:],
                                scalar1=mtl[:, c:c + 1],
                                scalar2=None,
                                op0=ALU.is_equal,
                            )
                            for h in range(nheads):
                                dst_ap = exG[:, c * NOC + h * C:
                                             c * NOC + (h + 1) * C]
                                src_ap = G[:, c * NOC + h * C:
                                           c * NOC + (h + 1) * C]
                                sc = exf[:, c * nheads + h:c * nheads + h + 1]
                                if h == nheads - 1 and nheads > 1:
                                    nc.scalar.activation(
                                        out=dst_ap, in_=src_ap, func=AF.Copy,
                                        scale=sc)
                                else:
                                    nc.vector.tensor_scalar(
                                        out=dst_ap, in0=src_ap, scalar1=sc,
                                        scalar2=None, op0=ALU.mult)
                            nc.tensor.matmul(
                                out=u_ps[:], lhsT=oh[:, c * 128:(c + 1) * 128],
                                rhs=exG[:, c * NOC:(c + 1) * NOC],
                                start=(c == 0), stop=(c == M - 1))
                            nc.tensor.matmul(
                                out=d_ps[:], lhsT=oh[:, c * 128:(c + 1) * 128],
                                rhs=exb[:, c * nheads:(c + 1) * nheads],
                                start=(c == 0), stop=(c == M - 1))
                        # normalize + mean over heads -> o_reg block
                        de = ep.tile([128, nheads], f32, tag="de")
                        nc.vector.tensor_scalar_add(out=de[:], in0=d_ps[:],
                                                    scalar1=1e-20)
                        dr = ep.tile([128, nheads], f32, tag="dr")
                        nc.vector.reciprocal(out=dr[:], in_=de[:])
                        if nheads > 1:
                            dq = ep.tile([128, nheads], f32, tag="dq")
                            nc.vector.tensor_scalar_mul(
                                out=dq[:], in0=dr[:], scalar1=1.0 / nheads)
                        else:
                            dq = dr
                        ob = o_reg[:, b * 128:b * 128 + C]
                        if last_cb is not None:
                            last_cb(b, u_ps, dq, ep)
                        else:
                            acc = ep.tile([128, C], f32, tag="acc")
                            t2 = ep.tile([128, C], f32, tag="t2")
                            nc.vector.tensor_scalar(
                                out=acc[:], in0=u_ps[:, 0:C],
                                scalar1=dq[:, 0:1], scalar2=None, op0=ALU.mult)
                            for h in range(1, nheads):
                                sc = dq[:, h:h + 1]
                                if h % 2 == 1:
                                    nc.scalar.activation(
                                        out=t2[:], in_=u_ps[:, h * C:(h + 1) * C],
                                        func=AF.Copy, scale=sc)
                                else:
                                    nc.vector.tensor_scalar(
                                        out=t2[:], in0=u_ps[:, h * C:(h + 1) * C],
                                        scalar1=sc, scalar2=None, op0=ALU.mult)
                                tgt = ob if h == nheads - 1 else acc
                                nc.vector.tensor_add(out=tgt, in0=acc[:],
                                                     in1=t2[:])
                        ch0 += M

            # ---------------- batchnorm + elu + transpose + allgather ----
            def bn_phase(layer, gcol, becol, st_in, st_out, xT_loc, xT_ag):
                with ExitStack() as st:
                    bp = st.enter_context(tc.tile_pool(name=f"b{layer}",
                                                       bufs=2))
                    pp = st.enter_context(
                        tc.tile_pool(name=f"bp{layer}", bufs=1, space="PSUM"))
                    tp = st.enter_context(
                        tc.tile_pool(name=f"bt{layer}", bufs=2, space="PSUM"))
                    # (pad rows of the last block are exactly 0: their onehot
                    # columns are all-zero so u=0 and 0 * (1/eps) = 0)
                    # per-(partition, channel) sums over blocks
                    acc = bp.tile([128, HID], f32, tag="acc")
                    nc.vector.reduce_sum(
                        out=acc[:],
                        in_=o_reg[:].rearrange("p (b c) -> p c b", c=128),
                        axis=mybir.AxisListType.X)
                    acc2 = bp.tile([128, HID], f32, tag="acc2")
                    sq = bp.tile([128, HID], f32, tag="sq")
                    for b in range(NBLK):
                        ob = o_reg[:, b * 128:b * 128 + HID]
                        nc.scalar.square(out=sq[:], in_=ob)
                        if b == 0:
                            nc.vector.tensor_copy(out=acc2[:], in_=sq[:])
                        else:
                            nc.vector.tensor_add(out=acc2[:], in0=acc2[:],
                                                 in1=sq[:])
                    # cross-partition reduce via matmul with ones
                    sp = pp.tile([128, 2], f32, space="PSUM", tag="sp")
                    nc.tensor.matmul(out=sp[:, 0:1], lhsT=acc[:], rhs=ones_s[:],
                                     start=True, stop=True)
                    nc.tensor.matmul(out=sp[:, 1:2], lhsT=acc2[:],
                                     rhs=ones_s[:], start=True, stop=True)
                    sts = bp.tile([128, 2], f32, tag="sts")
                    nc.vector.tensor_copy(out=sts[:], in_=sp[:])
                    nc.sync.dma_start(out=st_in[:], in_=sts[:])
                    nc.gpsimd.collective_compute(
                        "AllReduce", ALU.add, replica_groups=groups,
                        ins=[st_in.ap().opt()], outs=[st_out.ap().opt()])
                    stg = bp.tile([128, 2], f32, tag="stg")
                    nc.sync.dma_start(out=stg[:], in_=st_out[:])
                    # A = g*rsqrt(var+eps), B = be - mu*A   (per channel)
                    mu = bp.tile([128, 1], f32, tag="mu")
                    nc.vector.tensor_scalar_mul(out=mu[:], in0=stg[:, 0:1],
                                                scalar1=1.0 / N)
                    ms = bp.tile([128, 1], f32, tag="ms")
                    nc.vector.tensor_scalar_mul(out=ms[:], in0=stg[:, 1:2],
                                                scalar1=1.0 / N)
                    mu2 = bp.tile([128, 1], f32, tag="mu2")
                    nc.scalar.square(out=mu2[:], in_=mu[:])
                    var = bp.tile([128, 1], f32, tag="var")
                    nc.vector.tensor_sub(out=var[:], in0=ms[:], in1=mu2[:])
                    vare = bp.tile([128, 1], f32, tag="vare")
                    nc.vector.tensor_scalar_add(out=vare[:], in0=var[:],
                                                scalar1=BN_EPS)
                    sd = bp.tile([128, 1], f32, tag="sd")
                    nc.scalar.activation(out=sd[:], in_=vare[:], func=AF.Sqrt)
                    rs = bp.tile([128, 1], f32, tag="rs")
                    nc.vector.reciprocal(out=rs[:], in_=sd[:])
                    ab = bp.tile([128, 2], f32, tag="ab")
                    nc.vector.tensor_mul(out=ab[:, 0:1], in0=rs[:], in1=gcol)
                    tmp = bp.tile([128, 1], f32, tag="tmp1")
                    nc.vector.tensor_mul(out=tmp[:], in0=mu[:], in1=ab[:, 0:1])
                    nc.vector.tensor_sub(out=ab[:, 1:2], in0=becol, in1=tmp[:])
                    # transpose A and B columns separately -> [1,128] rows,
                    # then replicate to [128,128] each via K=1 matmuls
                    tA_ps = pp.tile([1, 128], f32, space="PSUM", tag="tA")
                    tB_ps = pp.tile([1, 128], f32, space="PSUM", tag="tB")
                    nc.tensor.transpose(out=tA_ps[:], in_=ab[:, 0:1],
                                        identity=ident_s[:])
                    nc.tensor.transpose(out=tB_ps[:], in_=ab[:, 1:2],
                                        identity=ident_s[:])
                    abT_a = bp.tile([1, 128], f32, tag="abTa")
                    abT_b = bp.tile([1, 128], f32, tag="abTb")
                    nc.vector.tensor_copy(out=abT_a[:], in_=tA_ps[:])
                    nc.vector.tensor_copy(out=abT_b[:], in_=tB_ps[:])
                    rep_ps = pp.tile([128, 256], f32, space="PSUM", tag="rep")
                    nc.tensor.matmul(out=rep_ps[:, 0:128],
                                     lhsT=onesr_s[:],
                                     rhs=abT_a[:], start=True, stop=True)
                    nc.tensor.matmul(out=rep_ps[:, 128:256],
                                     lhsT=onesr_s[:],
                                     rhs=abT_b[:], start=True, stop=True)
                    # per block: x = elu(o*A + B); write transposed bf16
                    for b in range(NBLK):
                        m = 128 if b < NBLK - 1 else LAST_M
                        ob = o_reg[:, b * 128:b * 128 + HID]
                        t = bp.tile([128, HID], f32, tag="bt")
                        nc.vector.tensor_mul(out=t[:], in0=ob,
                                             in1=rep_ps[:, 0:128])
                        t2 = bp.tile([128, HID], f32, tag="bt2")
                        nc.vector.tensor_add(out=t2[:], in0=t[:],
                                             in1=rep_ps[:, 128:256])
                        m0 = bp.tile([128, HID], f32, tag="bm0")
                        nc.vector.tensor_scalar_min(out=m0[:], in0=t2[:],
                                                    scalar1=0.0)
                        em = bp.tile([128, HID], f32, tag="bem")
                        nc.scalar.activation(out=em[:], in_=m0[:], func=AF.Exp)
                        r0 = bp.tile([128, HID], f32, tag="br0")
                        nc.vector.tensor_scalar(out=r0[:], in0=t2[:],
                                                scalar1=0.0, scalar2=-1.0,
                                                op0=ALU.max, op1=ALU.add)
                        xb = bp.tile([128, HID], f32, tag="bxb")
                        nc.vector.tensor_add(out=xb[:], in0=r0[:], in1=em[:])
                        tr_ps = tp.tile([128, 128], f32, space="PSUM",
                                        tag="tr")
                        nc.tensor.transpose(out=tr_ps[:], in_=xb[:],
                                            identity=ident_s[:])
                        xts = bp.tile([128, 128], dt_h, tag="xts")
                        nc.vector.tensor_copy(out=xts[:], in_=tr_ps[:])
                        nc.sync.dma_start(
                            out=xT_loc[:, b * 128:b * 128 + m],
                            in_=xts[:, :m])
                    nc.gpsimd.collective_compute(
                        "AllGather", ALU.bypass, replica_groups=groups,
                        ins=[xT_loc.ap().opt()], outs=[xT_ag.ap().opt()])

            # ---------------- layer 3 block finalizer --------------------
            def l3_final(b, u_ps, dq, ep):
                m = 128 if b < NBLK - 1 else LAST_M
                t = ep.tile([128, OUT], f32, tag="l3t")
                nc.vector.tensor_scalar(out=t[:], in0=u_ps[:, 0:OUT],
                                        scalar1=dq[:, 0:1], scalar2=None,
                                        op0=ALU.mult)
                o3 = ep.tile([128, OUT], f32, tag="l3o")
                nc.vector.tensor_add(out=o3[:], in0=t[:], in1=b3r_s[:])
                nc.sync.dma_start(out=out3[b * 128:b * 128 + m, :],
                                  in_=o3[:m])

            # ================= the network ===============================
            # barriers between phases consolidate cross-phase DMA waits
            # (walrus allows at most 2 sem waits per DMA instruction)
            dense(1, W1e_s, F_IN, HC, None, h1_rows, as1, ad1, HEADS)
            tc.strict_bb_all_engine_barrier()
            edge(1, h1_rows, as1, ad1, HEADS, HID)
            tc.strict_bb_all_engine_barrier()
            bn_phase(1, gbe_s[:, 0:1], gbe_s[:, 1:2], st1_in, st1_out,
                     x1T_loc, x1T_ag)
            tc.strict_bb_all_engine_barrier()
            dense(2, W2e_s, HID, HC, x1T_ag, h2_rows, as2, ad2, HEADS)
            tc.strict_bb_all_engine_barrier()
            edge(2, h2_rows, as2, ad2, HEADS, HID)
            tc.strict_bb_all_engine_barrier()
            bn_phase(2, gbe_s[:, 2:3], gbe_s[:, 3:4], st2_in, st2_out,
                     x2T_loc, x2T_ag)
            tc.strict_bb_all_engine_barrier()
            dense(3, W3e_s, HID, OUT, x2T_ag, h3_rows, as3, ad3, 1)
            tc.strict_bb_all_engine_barrier()
            edge(3, h3_rows, as3, ad3, 1, OUT, last_cb=l3_final)

    if fixup:
        _fixup_dma_waits(nc)
    return nc


# ---------------------------------------------------------------- host entry


def _prep_inputs(inputs, cfg):
    """Host-side preprocessing -> per-core input maps."""
    np_h = cfg.np_h
    x = np.asarray(inputs["x"], np.float32)
    ei = np.asarray(inputs["edge_index"], np.int64)
    N = cfg.N
    loop = np.arange(N, dtype=np.int64)
    src = np.concatenate([ei[0], loop])
    dst = np.concatenate([ei[1], loop])
    m_src, m_dst, m_dl, Mb = _edge_schedule(src, dst, cfg)

    W1e = _fold_alpha(np.asarray(inputs["W1"], np.float32),
                      np.asarray(inputs["as1"], np.float32),
                      np.asarray(inputs["ad1"], np.float32),
                      cfg.HEADS, cfg.HID)
    W2e = _fold_alpha(np.asarray(inputs["W2"], np.float32),
                      np.asarray(inputs["as2"], np.float32),
                      np.asarray(inputs["ad2"], np.float32),
                      cfg.HEADS, cfg.HID)
    W3e = _fold_alpha(np.asarray(inputs["W3"], np.float32),
                      np.asarray(inputs["as3"], np.float32),
                      np.asarray(inputs["ad3"], np.float32),
                      1, cfg.OUT)

    gbe = np.stack([np.asarray(inputs["g1"], np.float32),
                    np.asarray(inputs["be1"], np.float32),
                    np.asarray(inputs["g2"], np.float32),
                    np.asarray(inputs["be2"], np.float32)], axis=1)

    common = {
        "xT": np.ascontiguousarray(x.T).astype(np_h),
        "W1e": W1e.astype(np_h),
        "W2e": W2e.astype(np_h),
        "W3e": W3e.astype(np_h),
        "iota_i": np.tile(np.arange(128, dtype=np.float32), (128, 1)),
        "ident": np.eye(128, dtype=np.float32),
        "ones_c": np.ones((128, 1), np.float32),
        "ones_r": np.ones((1, 128), np.float32),
        "gbe": gbe.astype(np.float32),
        "b3r": np.tile(np.asarray(inputs["b3"], np.float32), (128, 1)),
    }
    in_maps = []
    for k in range(cfg.P):
        m = dict(common)
        m["m_src"] = m_src[k]
        m["m_dst"] = m_dst[k]
        m["m_dl"] = m_dl[k].astype(np.float32)
        in_maps.append(m)
    return in_maps, Mb


_CACHED = {}


def _get_program(cfg_key, cfg, Mb):
    key = (cfg_key, tuple(Mb.tolist()))
    if key not in _CACHED:
        _CACHED[key] = build_nc(cfg, Mb)
    return _CACHED[key]


def kernel(**inputs):
    cfg = Cfg()
    in_maps, Mb = _prep_inputs(inputs, cfg)
    nc = _get_program("full", cfg, Mb)
    res = run_bass_kernel_spmd(nc, in_maps, list(range(cfg.P)))
    shards = [res.results[k]["out3"][:cfg.NPC] for k in range(cfg.P)]
    return np.concatenate(shards, axis=0).astype(np.float32)


if __name__ == "__main__":
    # tiny smoke test of host prep only
    cfg = Cfg(N=1024, E=4096)
    rng = np.random.default_rng(0)
    src = rng.integers(0, cfg.N, cfg.E)
    dst = rng.integers(0, cfg.N, cfg.E)
    ms, md, mdl, Mb = _edge_schedule(
        np.concatenate([src, np.arange(cfg.N)]),
        np.concatenate([dst, np.arange(cfg.N)]), cfg)
    print("Mb:", Mb, "NCH:", Mb.sum())



# revision 3
# speedup vs baseline: 1.0549x; 1.0549x over previous
"""GAT (3-layer GATConv + BatchNorm + ELU) on Trainium2, 8 NeuronCores.

Input-space aggregation:
  out[d] = (1/H) sum_h [ (sum_{e->d} ex_eh * x[src_e]) / den_dh ] @ W_h
with ex = exp(leakyrelu(as[src] + ad[dst])), as = x @ (W_h a_s_h),
den = sum_e ex.  The dense per-node projection h = x@W is never
materialized; each dst block aggregates gathered x-rows with TensorE
one-hot matmuls (weighted one-hot built on DVE), then one small matmul
per head applies W.

Gathers use the SWDGE dma_gather custom instruction (int16 indices, 256B
row multiples): per superblock of SB dst blocks, 3 gathers (src rows with
index < 32768, src rows >= 32768, local ad rows).  Sharding: dst-node
range partitioning; per-layer features exchanged with an AllGather of
padded [NPC, 128/256]-col bf16 rows; ad values stay core-local.
"""

import sys

sys.path.insert(0, "/opt/trn_rl_repo")

import numpy as np
from contextlib import ExitStack

import concourse.bass as bass
import concourse.mybir as mybir
import concourse.tile as tile
from concourse.bass_utils import run_bass_kernel_spmd

AF = mybir.ActivationFunctionType
ALU = mybir.AluOpType
DT = mybir.dt

NEG_SLOPE = 0.2
BN_EPS = 1e-5
SPLIT = 32768            # int16 index range per gather table


class Cfg:
    def __init__(self, N=50000, E=400000, F_IN=64, HID=128, OUT=64, HEADS=4,
                 P=8, SB=5):
        assert N % P == 0
        self.N, self.E, self.F_IN, self.HID, self.OUT, self.HEADS, self.P = (
            N, E, F_IN, HID, OUT, HEADS, P,
        )
        self.NPC = N // P
        self.NBLK = (self.NPC + 127) // 128
        self.SB = SB
        self.NSUP = (self.NBLK + SB - 1) // SB
        self.dt_h = DT.bfloat16


# ---------------------------------------------------------------- host prep


def _wrap16(flat):
    """SWDGE idx layout: [128, n/16] i16, idx j at [j%16, j//16], 16-row
    pattern replicated 8x down the partitions."""
    n = len(flat)
    v = np.zeros((16, n // 16), np.int16)
    v[np.arange(n) % 16, np.arange(n) // 16] = flat
    return np.tile(v, (8, 1))


def _edge_schedule(src, dst, cfg):
    """Sort edges by dst; per core, per 128-dst block, split by src < SPLIT
    (lo) / >= SPLIT (hi); pad each half to a uniform (across cores) number
    of 128-edge chunks.  Chunk order: per superblock, all blocks' lo chunks
    then all blocks' hi chunks (so each gather's slot range is contiguous).

    Returns per-core (mDL [128,NCH] f32, LO16, HI16, AD16 wrapped idx
    arrays) plus (MbLO, MbHI) chunk counts."""
    P, NPC, NBLK, SB, NSUP = cfg.P, cfg.NPC, cfg.NBLK, cfg.SB, cfg.NSUP
    order = np.argsort(dst, kind="stable")
    s_s = src[order].astype(np.int64)
    d_s = dst[order].astype(np.int64)

    blk_starts = (np.arange(P)[:, None] * NPC + np.arange(NBLK)[None, :] * 128)
    bounds = np.searchsorted(d_s, blk_starts.reshape(-1))
    bounds = np.append(bounds, len(d_s))

    # per (core, block) lo/hi edge lists
    lo_e = {}
    hi_e = {}
    cnt_lo = np.zeros((P, NBLK), np.int64)
    cnt_hi = np.zeros((P, NBLK), np.int64)
    for k in range(P):
        for b in range(NBLK):
            i0 = bounds[k * NBLK + b]
            i1 = bounds[k * NBLK + b + 1] if (k * NBLK + b + 1) < P * NBLK \
                else bounds[-1]
            ss, dd = s_s[i0:i1], d_s[i0:i1]
            m = ss < SPLIT
            lo_e[(k, b)] = (ss[m], dd[m])
            hi_e[(k, b)] = (ss[~m], dd[~m])
            cnt_lo[k, b] = m.sum()
            cnt_hi[k, b] = (~m).sum()
    MbLO = (-(-cnt_lo // 128)).max(axis=0)
    MbHI = (-(-cnt_hi // 128)).max(axis=0)

    # global chunk numbering: per super: blocks' lo runs then hi runs, each
    # padded to a UNIFORM per-super count (so every dma_gather has the same
    # num_idxs -> few scalar registers).  Pad chunks are gathered (idx 0)
    # but never touched by compute.
    def _sup_sum(Mb):
        return [int(sum(Mb[b] for b in range(s * SB, min((s + 1) * SB, NBLK))))
                for s in range(NSUP)]
    PIECE = 8
    CHLOu = -(-max(_sup_sum(MbLO)) // PIECE) * PIECE
    CHHIu = -(-max(_sup_sum(MbHI)) // PIECE) * PIECE
    CHu = CHLOu + CHHIu
    loCH0 = np.zeros(NBLK, int)
    hiCH0 = np.zeros(NBLK, int)
    sup_ch0 = [s * CHu for s in range(NSUP)] + [NSUP * CHu]
    sup_lo = [CHLOu] * NSUP
    for s in range(NSUP):
        blks = range(s * SB, min((s + 1) * SB, NBLK))
        ch = s * CHu
        for b in blks:
            loCH0[b] = ch
            ch += MbLO[b]
        ch = s * CHu + CHLOu
        for b in blks:
            hiCH0[b] = ch
            ch += MbHI[b]
    NCH = NSUP * CHu

    mDL = np.full((P, NCH, 128), -1, np.float32)
    mLO = np.zeros((P, NCH, 128), np.int64)   # lo slots: src; pads 0
    mHI = np.zeros((P, NCH, 128), np.int64)   # hi slots: src-SPLIT; pads 0
    mAD = np.zeros((P, NCH, 128), np.int64)   # all slots: dst local; pads 0
    for k in range(P):
        for b in range(NBLK):
            base = (k * NPC + b * 128)
            for (ss, dd), ch0 in ((lo_e[(k, b)], loCH0[b]),
                                  (hi_e[(k, b)], hiCH0[b])):
                n = len(ss)
                if n == 0:
                    continue
                e0 = ch0 * 128
                mDL[k].reshape(-1)[e0:e0 + n] = (dd - base)
                sv = np.where(ss >= SPLIT, ss - SPLIT, ss)
                mLO[k].reshape(-1)[e0:e0 + n] = sv
                mHI[k].reshape(-1)[e0:e0 + n] = sv
                mAD[k].reshape(-1)[e0:e0 + n] = dd - k * NPC

    # wrapped int16 idx arrays, concatenated per super
    LO16, HI16, AD16 = [], [], []
    lo16_off, hi16_off, ad16_off = [0], [0], [0]
    for s in range(NSUP):
        c0s, c1s = sup_ch0[s], sup_ch0[s + 1]
        nlo = sup_lo[s]
        loflat = slice(c0s * 128, (c0s + nlo) * 128)
        hiflat = slice((c0s + nlo) * 128, c1s * 128)
        allflat = slice(c0s * 128, c1s * 128)
        LO16.append(np.stack([_wrap16(mLO[k].reshape(-1)[loflat])
                              for k in range(P)]))
        HI16.append(np.stack([_wrap16(mHI[k].reshape(-1)[hiflat])
                              for k in range(P)]))
        AD16.append(np.stack([_wrap16(mAD[k].reshape(-1)[allflat])
                              for k in range(P)]))
        lo16_off.append(lo16_off[-1] + LO16[-1].shape[2])
        hi16_off.append(hi16_off[-1] + HI16[-1].shape[2])
        ad16_off.append(ad16_off[-1] + AD16[-1].shape[2])
    LO16 = np.concatenate(LO16, axis=2)
    HI16 = np.concatenate(HI16, axis=2)
    AD16 = np.concatenate(AD16, axis=2)
    mDLt = np.ascontiguousarray(
        mDL.transpose(0, 2, 1)).astype(np.float32)  # [P, 128, NCH]

    meta = dict(MbLO=MbLO, MbHI=MbHI, loCH0=loCH0, hiCH0=hiCH0,
                sup_ch0=sup_ch0, sup_lo=sup_lo, NCH=NCH,
                lo16_off=lo16_off, hi16_off=hi16_off, ad16_off=ad16_off)
    return mDLt, LO16, HI16, AD16, meta


def _fold_asad(W, a_s, a_d, heads, ch):
    K = W.shape[0]
    Wr = W.reshape(K, heads, ch).astype(np.float64)
    Was = np.stack([Wr[:, h] @ a_s[h].astype(np.float64) for h in range(heads)],
                   axis=1)
    Wad = np.stack([Wr[:, h] @ a_d[h].astype(np.float64) for h in range(heads)],
                   axis=1)
    return Was.astype(np.float32), Wad.astype(np.float32)


# ---------------------------------------------------------------- device


def _fixup_dma_waits(nc, max_waits=1):
    """Move excess semaphore waits onto NoOps (walrus sync-wait limit)."""
    cnt = 0
    for bb in nc.main_func.blocks:
        new = []
        for inst in bb.instructions:
            si = getattr(inst, "sync_info", None)
            if (
                "Branch" not in type(inst).__name__
                and si is not None
                and si.on_wait
                and len(si.on_wait) > max_waits
            ):
                extra = list(si.on_wait[:-max_waits])
                keep = list(si.on_wait[-max_waits:])
                for w in extra:
                    nop = mybir.InstNoOp(
                        name=f"I-dmaw-{cnt}",
                        sync_info=mybir.SyncInfo(on_wait=[w], on_update=[]),
                        bass_nofuse=True,
                        engine=inst.engine,
                    )
                    cnt += 1
                    new.append(nop)
                inst.sync_info = mybir.SyncInfo(
                    on_wait=keep, on_update=list(si.on_update))
            new.append(inst)
        bb.instructions[:] = new
    return cnt


def build_nc(cfg, meta, num_cores=None, fixup=True):
    P, N, NPC, NBLK, SB, NSUP = (cfg.P, cfg.N, cfg.NPC, cfg.NBLK, cfg.SB,
                                 cfg.NSUP)
    HEADS, HID, OUT, F_IN = cfg.HEADS, cfg.HID, cfg.OUT, cfg.F_IN
    dt_h = cfg.dt_h
    f32 = DT.float32
    NCH = meta["NCH"]
    MbLO, MbHI = meta["MbLO"], meta["MbHI"]
    loCH0, hiCH0 = meta["loCH0"], meta["hiCH0"]
    sup_ch0, sup_lo = meta["sup_ch0"], meta["sup_lo"]
    lo16_off, hi16_off, ad16_off = (meta["lo16_off"], meta["hi16_off"],
                                    meta["ad16_off"])
    LAST_M = NPC - (NBLK - 1) * 128

    EW1, EW2, EW3 = 128, 256, 256     # padded gather row cols (bf16)
    ADW = 128                         # padded ad row cols (bf16)
    FE1, FE2, FE3 = F_IN + HEADS, HID + HEADS, HID + 1

    nc = bass.Bass(trn_type="TRN2", num_devices=(num_cores or P),
                   dynamic_dma_scratch_size=32768)

    # ---- parameters
    xext1 = nc.declare_dram_parameter("xext1", [N, EW1], dt_h, isOutput=False)
    ad1_l = nc.declare_dram_parameter("ad1_l", [NPC, ADW], dt_h, isOutput=False)
    pDL = nc.declare_dram_parameter("mDL", [128, NCH], f32, isOutput=False)
    pLO = nc.declare_dram_parameter("LO16", [128, lo16_off[-1]], DT.int16,
                                    isOutput=False)
    pHI = nc.declare_dram_parameter("HI16", [128, hi16_off[-1]], DT.int16,
                                    isOutput=False)
    pAD = nc.declare_dram_parameter("AD16", [128, ad16_off[-1]], DT.int16,
                                    isOutput=False)
    pW1 = nc.declare_dram_parameter("W1t", [F_IN, HEADS * HID], dt_h, isOutput=False)
    pW2 = nc.declare_dram_parameter("W2t", [HID, HEADS * HID], dt_h, isOutput=False)
    pW3 = nc.declare_dram_parameter("W3t", [HID, OUT], dt_h, isOutput=False)
    pWasd2 = nc.declare_dram_parameter("Wasd2", [HID, 2 * HEADS], dt_h, isOutput=False)
    pWasd3 = nc.declare_dram_parameter("Wasd3", [HID, 2], dt_h, isOutput=False)
    iota_i = nc.declare_dram_parameter("iota_i", [128, 128], f32, isOutput=False)
    ident = nc.declare_dram_parameter("ident", [128, 128], f32, isOutput=False)
    ones_c = nc.declare_dram_parameter("ones_c", [128, 1], f32, isOutput=False)
    ones_r = nc.declare_dram_parameter("ones_r", [1, 128], f32, isOutput=False)
    gbe = nc.declare_dram_parameter("gbe", [128, 4], f32, isOutput=False)
    b3r = nc.declare_dram_parameter("b3r", [128, OUT], f32, isOutput=False)
    out3 = nc.declare_dram_parameter("out3", [NBLK * 128, OUT], f32, isOutput=True)

    # ---- internal DRAM
    xo2 = nc.dram_tensor("xo2", [NPC, EW2], dt_h)
    xo3 = nc.dram_tensor("xo3", [NPC, EW3], dt_h)
    xa2 = nc.dram_tensor("xa2", [P, NPC, EW2], dt_h, addr_space="Shared")
    xa3 = nc.dram_tensor("xa3", [P, NPC, EW3], dt_h, addr_space="Shared")
    ad2_l = nc.dram_tensor("ad2_l", [NPC, ADW], dt_h)
    ad3_l = nc.dram_tensor("ad3_l", [NPC, ADW], dt_h)
    st_in = [nc.dram_tensor(f"st{i}_in", [128, 2], f32) for i in (1, 2)]
    st_out = [nc.dram_tensor(f"st{i}_out", [128, 2], f32, addr_space="Shared")
              for i in (1, 2)]

    groups = [list(range(P))]

    def sup_blocks(s):
        return range(s * SB, min((s + 1) * SB, NBLK))

    from concourse.library_config import mlp as _mlp_lib
    CHLOu = int(sup_lo[0])
    CHu = int(sup_ch0[1] - sup_ch0[0])
    CHHIu = CHu - CHLOu
    with tile.TileContext(nc, num_cores=(num_cores or P)) as tc:
        with ExitStack() as top:
            nc.gpsimd.load_library(_mlp_lib)
            PIECE = 8
            r_pc = nc.alloc_register(mybir.EngineType.Pool, "r_pc")
            nc.gpsimd.reg_mov(r_pc, PIECE * 128)
            cpool = top.enter_context(tc.tile_pool(name="consts", bufs=1))
            mpool = top.enter_context(tc.tile_pool(name="meta", bufs=1))
            opool = top.enter_context(tc.tile_pool(name="oreg", bufs=1))

            def cload(name, shape, dtype, src):
                t = cpool.tile(shape, dtype, tag=name)
                nc.sync.dma_start(out=t[:], in_=src)
                return t

            iota_s = cload("iota", [128, 128], f32, iota_i[:])
            ident_s = cload("ident", [128, 128], f32, ident[:])
            ones_s = cload("ones", [128, 1], f32, ones_c[:])
            onesr_s = cload("onesr", [1, 128], f32, ones_r[:])
            gbe_s = cload("gbe", [128, 4], f32, gbe[:])
            b3r_s = cload("b3r", [128, OUT], f32, b3r[:])
            W1_s = cload("w1", [F_IN, HEADS * HID], dt_h, pW1[:])
            W2_s = cload("w2", [HID, HEADS * HID], dt_h, pW2[:])
            W3_s = cload("w3", [HID, OUT], dt_h, pW3[:])
            Wasd2_s = cload("wasd2", [HID, 2 * HEADS], dt_h, pWasd2[:])
            Wasd3_s = cload("wasd3", [HID, 2], dt_h, pWasd3[:])

            mDL_s = mpool.tile([128, NCH], f32, tag="mdl")
            nc.sync.dma_start(out=mDL_s[:], in_=pDL[:])
            LO_s = mpool.tile([128, lo16_off[-1]], DT.int16, tag="lo16")
            nc.sync.dma_start(out=LO_s[:], in_=pLO[:])
            HI_s = mpool.tile([128, hi16_off[-1]], DT.int16, tag="hi16")
            nc.sync.dma_start(out=HI_s[:], in_=pHI[:])
            AD_s = mpool.tile([128, ad16_off[-1]], DT.int16, tag="ad16")
            nc.sync.dma_start(out=AD_s[:], in_=pAD[:])

            o_reg = opool.tile([128, NBLK * 128], f32, tag="oreg")

            # ============== edge phase ==============
            def edge(layer, xtab_lo, xtab_hi, adtab, EW, F, nh, W_s, NO,
                     stats_ps=None, l3_sup_cb=None):
                with ExitStack() as st:
                    gp = st.enter_context(tc.tile_pool(name=f"g{layer}", bufs=2))
                    wp = st.enter_context(tc.tile_pool(name=f"w{layer}", bufs=3))
                    up = st.enter_context(
                        tc.tile_pool(name=f"u{layer}", bufs=2, space="PSUM"))
                    zp = st.enter_context(
                        tc.tile_pool(name=f"z{layer}", bufs=1, space="PSUM"))
                    for s in range(NSUP):
                        blks = list(sup_blocks(s))
                        ch0, ch1 = int(sup_ch0[s]), int(sup_ch0[s + 1])
                        CH = ch1 - ch0
                        CHLO = int(sup_lo[s])
                        CHHI = CH - CHLO
                        # ---- super gathers (SWDGE dma_gather)
                        G = gp.tile([128, CH * EW], dt_h, tag="G")
                        G3 = G[:].rearrange("p (m w) -> p m w", w=EW)
                        for p0 in range(0, CHLO, PIECE):
                            nc.gpsimd.dma_gather(
                                out_ap=G3[:, p0:p0 + PIECE, :],
                                in_ap=xtab_lo,
                                idxs_ap=LO_s[:, lo16_off[s] + p0 * 8:
                                             lo16_off[s] + (p0 + PIECE) * 8],
                                num_idxs=PIECE * 128,
                                num_idxs_reg=r_pc,
                                elem_size=EW)
                        for p0 in range(0, CHHI, PIECE):
                            nc.gpsimd.dma_gather(
                                out_ap=G3[:, CHLO + p0:CHLO + p0 + PIECE, :],
                                in_ap=xtab_hi,
                                idxs_ap=HI_s[:, hi16_off[s] + p0 * 8:
                                             hi16_off[s] + (p0 + PIECE) * 8],
                                num_idxs=PIECE * 128,
                                num_idxs_reg=r_pc,
                                elem_size=EW)
                        Ad = gp.tile([128, CH * ADW], dt_h, tag="Ad")
                        Ad3 = Ad[:].rearrange("p (m w) -> p m w", w=ADW)
                        for p0 in range(0, CH, PIECE):
                            nc.gpsimd.dma_gather(
                                out_ap=Ad3[:, p0:p0 + PIECE, :],
                                in_ap=adtab,
                                idxs_ap=AD_s[:, ad16_off[s] + p0 * 8:
                                             ad16_off[s] + (p0 + PIECE) * 8],
                                num_idxs=PIECE * 128,
                                num_idxs_reg=r_pc,
                                elem_size=ADW)
                        # ---- ex = exp(leakyrelu(as+ad)) for whole super
                        v = gp.tile([128, CH * nh], f32, tag="v")
                        nc.vector.tensor_tensor(
                            out=v[:].rearrange("p (m w) -> p m w", w=nh),
                            in0=G3[:, :, F:F + nh],
                            in1=Ad[:].rearrange(
                                "p (m w) -> p m w", w=ADW)[:, :, 0:nh],
                            op=ALU.add)
                        lr = gp.tile([128, CH * nh], f32, tag="lrt")
                        nc.scalar.activation(out=lr[:], in_=v[:], func=AF.Prelu,
                                             alpha=NEG_SLOPE)
                        exf = gp.tile([128, CH * nh], f32, tag="exf")
                        nc.scalar.activation(out=exf[:], in_=lr[:], func=AF.Exp)
                        exb = gp.tile([128, CH * nh], dt_h, tag="exb")
                        nc.vector.tensor_copy(out=exb[:], in_=exf[:])

                        if l3_sup_cb is not None:
                            osup = wp.tile([128, SB * OUT], f32, tag="osup")

                        for b in blks:
                            chunks = (
                                list(range(loCH0[b], loCH0[b] + MbLO[b]))
                                + list(range(hiCH0[b], hiCH0[b] + MbHI[b])))
                            uT = up.tile([F, nh * 128], f32, space="PSUM",
                                         tag="uT")
                            denT = up.tile([nh, 128], f32, space="PSUM",
                                           tag="denT")
                            for ci, c in enumerate(chunks):
                                cl = c - ch0
                                ohw = wp.tile([128, nh * 128], dt_h, tag="ohw")
                                oh = wp.tile([128, 128], dt_h, tag="oh")
                                nc.vector.tensor_scalar(
                                    out=oh[:], in0=iota_s[:],
                                    scalar1=mDL_s[:, c:c + 1], scalar2=None,
                                    op0=ALU.is_equal)
                                for h in range(nh):
                                    sc = exf[:, cl * nh + h:cl * nh + h + 1]
                                    dst = ohw[:, h * 128:(h + 1) * 128]
                                    if h % 2 == 1:
                                        nc.scalar.activation(
                                            out=dst, in_=oh[:], func=AF.Copy,
                                            scale=sc)
                                    else:
                                        nc.vector.tensor_scalar(
                                            out=dst, in0=oh[:], scalar1=sc,
                                            scalar2=None, op0=ALU.mult)
                                first = ci == 0
                                last = ci == len(chunks) - 1
                                nc.tensor.matmul(
                                    out=uT[:], lhsT=G3[:, cl, 0:F], rhs=ohw[:],
                                    start=first, stop=last)
                                nc.tensor.matmul(
                                    out=denT[:],
                                    lhsT=exb[:, cl * nh:(cl + 1) * nh],
                                    rhs=oh[:],
                                    start=first, stop=last)
                            # ---- block epilogue
                            de = wp.tile([nh, 128], f32, tag="de")
                            nc.vector.tensor_scalar_add(
                                out=de[:], in0=denT[:], scalar1=1e-20)
                            dq_ps = zp.tile([128, nh], f32, space="PSUM",
                                            tag="dqps")
                            nc.tensor.transpose(out=dq_ps[:], in_=de[:],
                                                identity=ident_s[0:nh, 0:nh])
                            dq = wp.tile([128, nh], f32, tag="dq")
                            nc.vector.reciprocal(out=dq[:], in_=dq_ps[:])
                            uTn = wp.tile([F, nh * 128], dt_h, tag="uTn")
                            nc.vector.tensor_copy(out=uTn[:], in_=uT[:])
                            z_ps = zp.tile([128, nh * NO], f32, space="PSUM",
                                           tag="zps")
                            for h in range(nh):
                                nc.tensor.matmul(
                                    out=z_ps[:, h * NO:(h + 1) * NO],
                                    lhsT=uTn[:, h * 128:(h + 1) * 128],
                                    rhs=W_s[:, h * NO:(h + 1) * NO],
                                    start=True, stop=True)
                            if l3_sup_cb is None:
                                ob = o_reg[:, b * 128:b * 128 + NO]
                            else:
                                ob = osup[:, (b - blks[0]) * OUT:
                                          (b - blks[0] + 1) * OUT]
                            if nh > 1:
                                acc = wp.tile([128, NO], f32, tag="acc")
                                t2 = wp.tile([128, NO], f32, tag="t2")
                            nc.vector.tensor_scalar(
                                out=(acc[:] if nh > 1 else ob),
                                in0=z_ps[:, 0:NO],
                                scalar1=dq[:, 0:1], scalar2=None, op0=ALU.mult)
                            for h in range(1, nh):
                                sc = dq[:, h:h + 1]
                                if h % 2 == 1:
                                    nc.scalar.activation(
                                        out=t2[:],
                                        in_=z_ps[:, h * NO:(h + 1) * NO],
                                        func=AF.Copy, scale=sc)
                                else:
                                    nc.vector.tensor_scalar(
                                        out=t2[:],
                                        in0=z_ps[:, h * NO:(h + 1) * NO],
                                        scalar1=sc, scalar2=None, op0=ALU.mult)
                                tgt = ob if h == nh - 1 else acc
                                nc.vector.tensor_add(out=tgt, in0=acc[:],
                                                     in1=t2[:])
                            if stats_ps is not None:
                                sq = wp.tile([128, NO], f32, tag="sq")
                                nc.scalar.activation(out=sq[:], in_=ob,
                                                     func=AF.Square)
                                nc.tensor.matmul(
                                    out=stats_ps[:, 0:1], lhsT=ob,
                                    rhs=ones_s[:],
                                    start=(b == 0), stop=(b == NBLK - 1))
                                nc.tensor.matmul(
                                    out=stats_ps[:, 1:2], lhsT=sq[:],
                                    rhs=ones_s[:],
                                    start=(b == 0), stop=(b == NBLK - 1))
                        if l3_sup_cb is not None:
                            l3_sup_cb(s, blks, osup)

            # ============== bn + elu + pack + allgather ==============
            def bn_phase(li, gcol, becol, sti, sto, Wasd, EW, FE, nh2, xo, xa,
                         ad_next, stats_ps):
                with ExitStack() as st:
                    bp = st.enter_context(tc.tile_pool(name=f"b{li}", bufs=2))
                    pp = st.enter_context(
                        tc.tile_pool(name=f"bp{li}", bufs=1, space="PSUM"))
                    pq = st.enter_context(
                        tc.tile_pool(name=f"bq{li}", bufs=2, space="PSUM"))
                    sts = bp.tile([128, 2], f32, tag="sts")
                    nc.vector.tensor_copy(out=sts[:], in_=stats_ps[:])
                    nc.sync.dma_start(out=sti[:], in_=sts[:])
                    nc.gpsimd.collective_compute(
                        "AllReduce", ALU.add, replica_groups=groups,
                        ins=[sti.ap().opt()], outs=[sto.ap().opt()])
                    stg = bp.tile([128, 2], f32, tag="stg")
                    nc.sync.dma_start(out=stg[:], in_=sto[:])
                    mu = bp.tile([128, 1], f32, tag="mu")
                    nc.vector.tensor_scalar_mul(out=mu[:], in0=stg[:, 0:1],
                                                scalar1=1.0 / N)
                    ms = bp.tile([128, 1], f32, tag="ms")
                    nc.vector.tensor_scalar_mul(out=ms[:], in0=stg[:, 1:2],
                                                scalar1=1.0 / N)
                    mu2 = bp.tile([128, 1], f32, tag="mu2")
                    nc.scalar.activation(out=mu2[:], in_=mu[:], func=AF.Square)
                    var = bp.tile([128, 1], f32, tag="var")
                    nc.vector.tensor_sub(out=var[:], in0=ms[:], in1=mu2[:])
                    vare = bp.tile([128, 1], f32, tag="vare")
                    nc.vector.tensor_scalar_add(out=vare[:], in0=var[:],
                                                scalar1=BN_EPS)
                    sd = bp.tile([128, 1], f32, tag="sd")
                    nc.scalar.activation(out=sd[:], in_=vare[:], func=AF.Sqrt)
                    rs = bp.tile([128, 1], f32, tag="rs")
                    nc.vector.reciprocal(out=rs[:], in_=sd[:])
                    ab = bp.tile([128, 2], f32, tag="ab")
                    nc.vector.tensor_mul(out=ab[:, 0:1], in0=rs[:], in1=gcol)
                    tmp = bp.tile([128, 1], f32, tag="tmp1")
                    nc.vector.tensor_mul(out=tmp[:], in0=mu[:], in1=ab[:, 0:1])
                    nc.vector.tensor_sub(out=ab[:, 1:2], in0=becol, in1=tmp[:])
                    tA_ps = pp.tile([1, 128], f32, space="PSUM", tag="tA")
                    tB_ps = pp.tile([1, 128], f32, space="PSUM", tag="tB")
                    nc.tensor.transpose(out=tA_ps[:], in_=ab[:, 0:1],
                                        identity=ident_s[:])
                    nc.tensor.transpose(out=tB_ps[:], in_=ab[:, 1:2],
                                        identity=ident_s[:])
                    abT_a = bp.tile([1, 128], f32, tag="abTa")
                    abT_b = bp.tile([1, 128], f32, tag="abTb")
                    nc.vector.tensor_copy(out=abT_a[:], in_=tA_ps[:])
                    nc.vector.tensor_copy(out=abT_b[:], in_=tB_ps[:])
                    rep_ps = pp.tile([128, 256], f32, space="PSUM", tag="rep")
                    nc.tensor.matmul(out=rep_ps[:, 0:128], lhsT=onesr_s[:],
                                     rhs=abT_a[:], start=True, stop=True)
                    nc.tensor.matmul(out=rep_ps[:, 128:256], lhsT=onesr_s[:],
                                     rhs=abT_b[:], start=True, stop=True)
                    rep = bp.tile([128, 256], f32, tag="repc")
                    nc.vector.tensor_copy(out=rep[:], in_=rep_ps[:])

                    for s in range(NSUP):
                        blks = list(sup_blocks(s))
                        xrow = bp.tile([128, SB * FE], dt_h, tag="xrow")
                        adn = bp.tile([128, SB * nh2], dt_h, tag="adn")
                        for b in blks:
                            j = b - blks[0]
                            zb = o_reg[:, b * 128:b * 128 + HID]
                            t = bp.tile([128, HID], f32, tag="bt")
                            nc.vector.tensor_mul(out=t[:], in0=zb,
                                                 in1=rep[:, 0:128])
                            t2 = bp.tile([128, HID], f32, tag="bt2")
                            nc.vector.tensor_add(out=t2[:], in0=t[:],
                                                 in1=rep[:, 128:256])
                            m0 = bp.tile([128, HID], f32, tag="bm0")
                            nc.vector.tensor_scalar_min(out=m0[:], in0=t2[:],
                                                        scalar1=0.0)
                            em = bp.tile([128, HID], f32, tag="bem")
                            nc.scalar.activation(out=em[:], in_=m0[:],
                                                 func=AF.Exp)
                            r0 = bp.tile([128, HID], f32, tag="br0")
                            nc.vector.tensor_scalar(out=r0[:], in0=t2[:],
                                                    scalar1=0.0, scalar2=-1.0,
                                                    op0=ALU.max, op1=ALU.add)
                            xbf = bp.tile([128, HID], f32, tag="xbf")
                            nc.vector.tensor_add(out=xbf[:], in0=r0[:],
                                                 in1=em[:])
                            xb = xrow[:, j * FE:j * FE + HID]
                            nc.scalar.copy(out=xb, in_=xbf[:])
                            xT_ps = pq.tile([128, 128], f32, space="PSUM",
                                            tag="xTps")
                            nc.tensor.transpose(
                                out=xT_ps[:], in_=xbf[:], identity=ident_s[:])
                            xT_s = bp.tile([128, 128], dt_h, tag="xTs")
                            nc.vector.tensor_copy(out=xT_s[:], in_=xT_ps[:])
                            aa_ps = pq.tile([128, 2 * nh2], f32, space="PSUM",
                                            tag="aaps")
                            nc.tensor.matmul(out=aa_ps[:], lhsT=xT_s[:],
                                             rhs=Wasd[:], start=True, stop=True)
                            nc.vector.tensor_copy(
                                out=xrow[:, j * FE + HID:(j + 1) * FE],
                                in_=aa_ps[:, 0:nh2])
                            nc.vector.tensor_copy(
                                out=adn[:, j * nh2:(j + 1) * nh2],
                                in_=aa_ps[:, nh2:2 * nh2])
                        # write super's rows (ragged last block separately)
                        nb = len(blks)
                        r0_ = blks[0] * 128
                        xr3 = xrow[:].rearrange("p (b w) -> p b w", w=FE)
                        ad3v = adn[:].rearrange("p (b w) -> p b w", w=nh2)
                        if blks[-1] == NBLK - 1:
                            nfull = nb - 1
                            if nfull:
                                nc.sync.dma_start(
                                    out=xo[r0_:r0_ + nfull * 128, 0:FE]
                                    .rearrange("(b d) w -> d b w", d=128),
                                    in_=xr3[:, 0:nfull, :])
                                nc.sync.dma_start(
                                    out=ad_next[r0_:r0_ + nfull * 128, 0:nh2]
                                    .rearrange("(b d) w -> d b w", d=128),
                                    in_=ad3v[:, 0:nfull, :])
                            lb0 = (NBLK - 1) * 128
                            nc.sync.dma_start(
                                out=xo[lb0:lb0 + LAST_M, 0:FE],
                                in_=xrow[0:LAST_M, (nb - 1) * FE:nb * FE])
                            nc.sync.dma_start(
                                out=ad_next[lb0:lb0 + LAST_M, 0:nh2],
                                in_=adn[0:LAST_M, (nb - 1) * nh2:nb * nh2])
                        else:
                            nc.sync.dma_start(
                                out=xo[r0_:r0_ + nb * 128, 0:FE]
                                .rearrange("(b d) w -> d b w", d=128),
                                in_=xr3[:, 0:nb, :])
                            nc.sync.dma_start(
                                out=ad_next[r0_:r0_ + nb * 128, 0:nh2]
                                .rearrange("(b d) w -> d b w", d=128),
                                in_=ad3v[:, 0:nb, :])
                    nc.gpsimd.collective_compute(
                        "AllGather", ALU.bypass, replica_groups=groups,
                        ins=[xo.ap().opt()], outs=[xa.ap().opt()])

            # ============== layer 3 final writeout ==============
            def l3_final(s, blks, osup):
                nb = len(blks)
                r0_ = blks[0] * 128
                for j in range(nb):
                    sl = osup[:, j * OUT:(j + 1) * OUT]
                    nc.vector.tensor_add(out=sl, in0=sl, in1=b3r_s[:, 0:OUT])
                ob3 = osup[:].rearrange("p (b w) -> p b w", w=OUT)
                if blks[-1] == NBLK - 1:
                    nfull = nb - 1
                    if nfull:
                        nc.sync.dma_start(
                            out=out3[r0_:r0_ + nfull * 128, :].rearrange(
                                "(b d) w -> d b w", d=128),
                            in_=ob3[:, 0:nfull, :])
                    lb0 = (NBLK - 1) * 128
                    nc.sync.dma_start(
                        out=out3[lb0:lb0 + LAST_M, :],
                        in_=osup[0:LAST_M, (nb - 1) * OUT:nb * OUT])
                else:
                    nc.sync.dma_start(
                        out=out3[r0_:r0_ + nb * 128, :].rearrange(
                            "(b d) w -> d b w", d=128),
                        in_=ob3[:, 0:nb, :])

            # ================= the network =================
            spool = top.enter_context(tc.tile_pool(name="stats", bufs=1,
                                                   space="PSUM"))
            st1_ps = spool.tile([128, 2], f32, space="PSUM", tag="st")

            edge(1, xext1[:, :], xext1[SPLIT:N, :], ad1_l[:, :], EW1, F_IN,
                 HEADS, W1_s, HID, stats_ps=st1_ps)
            tc.strict_bb_all_engine_barrier()
            bn_phase(1, gbe_s[:, 0:1], gbe_s[:, 1:2], st_in[0], st_out[0],
                     Wasd2_s, EW2, FE2, HEADS, xo2, xa2, ad2_l, st1_ps)
            tc.strict_bb_all_engine_barrier()
            st2_ps = spool.tile([128, 2], f32, space="PSUM", tag="st")
            xa2f = xa2.rearrange("p n w -> (p n) w")
            edge(2, xa2f[:, :], xa2f[SPLIT:N, :], ad2_l[:, :], EW2, HID,
                 HEADS, W2_s, HID, stats_ps=st2_ps)
            tc.strict_bb_all_engine_barrier()
            bn_phase(2, gbe_s[:, 2:3], gbe_s[:, 3:4], st_in[1], st_out[1],
                     Wasd3_s, EW3, FE3, 1, xo3, xa3, ad3_l, st2_ps)
            tc.strict_bb_all_engine_barrier()
            xa3f = xa3.rearrange("p n w -> (p n) w")
            edge(3, xa3f[:, :], xa3f[SPLIT:N, :], ad3_l[:, :], EW3, HID, 1,
                 W3_s, OUT, l3_sup_cb=l3_final)

    if fixup:
        mybir.codegen_inst_isa_subclasses(nc)
    _fixup_dma_waits(nc)
    return nc


# ---------------------------------------------------------------- host entry


def _prep_inputs(inputs, cfg):
    import ml_dtypes
    bf16 = ml_dtypes.bfloat16
    x = np.asarray(inputs["x"], np.float32)
    ei = np.asarray(inputs["edge_index"], np.int64)
    N, P, NPC = cfg.N, cfg.P, cfg.NPC
    loop = np.arange(N, dtype=np.int64)
    src = np.concatenate([ei[0], loop])
    dst = np.concatenate([ei[1], loop])
    mDLt, LO16, HI16, AD16, meta = _edge_schedule(src, dst, cfg)

    W1 = np.asarray(inputs["W1"], np.float32)
    W2 = np.asarray(inputs["W2"], np.float32)
    W3 = np.asarray(inputs["W3"], np.float32)
    Was1, Wad1 = _fold_asad(W1, np.asarray(inputs["as1"], np.float32),
                            np.asarray(inputs["ad1"], np.float32),
                            cfg.HEADS, cfg.HID)
    Was2, Wad2 = _fold_asad(W2, np.asarray(inputs["as2"], np.float32),
                            np.asarray(inputs["ad2"], np.float32),
                            cfg.HEADS, cfg.HID)
    Was3, Wad3 = _fold_asad(W3, np.asarray(inputs["as3"], np.float32),
                            np.asarray(inputs["ad3"], np.float32),
                            1, cfg.OUT)

    as1 = (x @ Was1).astype(np.float32)
    ad1 = (x @ Wad1).astype(np.float32)
    xext1 = np.zeros((N, 128), np.float32)
    xext1[:, :cfg.F_IN] = x
    xext1[:, cfg.F_IN:cfg.F_IN + cfg.HEADS] = as1

    gbe_ = np.stack([np.asarray(inputs["g1"], np.float32),
                     np.asarray(inputs["be1"], np.float32),
                     np.asarray(inputs["g2"], np.float32),
                     np.asarray(inputs["be2"], np.float32)], axis=1)

    common = {
        "xext1": xext1.astype(bf16),
        "mDL": None, "LO16": None, "HI16": None, "AD16": None,
        "W1t": (W1 / cfg.HEADS).astype(bf16),
        "W2t": (W2 / cfg.HEADS).astype(bf16),
        "W3t": W3.astype(bf16),
        "Wasd2": np.concatenate([Was2, Wad2], axis=1).astype(bf16),
        "Wasd3": np.concatenate([Was3, Wad3], axis=1).astype(bf16),
        "iota_i": np.tile(np.arange(128, dtype=np.float32), (128, 1)),
        "ident": np.eye(128, dtype=np.float32),
        "ones_c": np.ones((128, 1), np.float32),
        "ones_r": np.ones((1, 128), np.float32),
        "gbe": gbe_.astype(np.float32),
        "b3r": np.tile(np.asarray(inputs["b3"], np.float32), (128, 1)),
    }
    in_maps = []
    for k in range(P):
        m = dict(common)
        m["mDL"] = mDLt[k]
        m["LO16"] = np.ascontiguousarray(LO16[k])
        m["HI16"] = np.ascontiguousarray(HI16[k])
        m["AD16"] = np.ascontiguousarray(AD16[k])
        adl = np.zeros((NPC, 128), np.float32)
        adl[:, :cfg.HEADS] = ad1[k * NPC:(k + 1) * NPC]
        m["ad1_l"] = adl.astype(bf16)
        in_maps.append(m)
    return in_maps, meta


_CACHED = {}


def _get_program(cfg_key, cfg, meta):
    key = (cfg_key, tuple(meta["MbLO"].tolist()), tuple(meta["MbHI"].tolist()))
    if key not in _CACHED:
        _CACHED[key] = build_nc(cfg, meta)
    return _CACHED[key]


def kernel(**inputs):
    cfg = Cfg()
    in_maps, meta = _prep_inputs(inputs, cfg)
    nc = _get_program("full", cfg, meta)
    res = run_bass_kernel_spmd(nc, in_maps, list(range(cfg.P)))
    shards = [res.results[k]["out3"][:cfg.NPC] for k in range(cfg.P)]
    return np.concatenate(shards, axis=0).astype(np.float32)


if __name__ == "__main__":
    cfg = Cfg()
    rng = np.random.default_rng(0)
    src = np.concatenate([rng.integers(0, cfg.N, cfg.E), np.arange(cfg.N)])
    dst = np.concatenate([rng.integers(0, cfg.N, cfg.E), np.arange(cfg.N)])
    mDLt, LO16, HI16, AD16, meta = _edge_schedule(src, dst, cfg)
    print("NCH:", meta["NCH"], "MbLO[:8]:", meta["MbLO"][:8],
          "MbHI[:8]:", meta["MbHI"][:8])
